# revision 1
# baseline (speedup 1.0000x reference)
"""CTBG circuit kernel for Trainium2, data-parallel over batch on 8 NeuronCores.

Network (per reference):
  gpe_out = x @ (gpe_w * gpe_mask.T) + gpe_b              [B, 1536]
  gpi_in  = concat([x, gpe_out], -1)                      [B, 3072]
  gpi_out = gpi_in @ (gpi_w * gpi_mask.T) + gpi_b         [B, 1536]
  h1 = relu(gpi_out @ w1 + b1); h2 = relu(h1 @ w2 + b2)
  out = relu(h2 @ w3 + b3)                                [B, 6]

Key algebraic identity: gpe_out and gpi_out feed forward with no
intervening nonlinearity, so the masked front end folds into one
[1536, 512] weight computed ON DEVICE once per launch:

  mw_gpe = gpe_w * gpe_mask.T
  mw_gpi = gpi_w * gpi_mask.T
  M      = mw_gpi[1536:] @ w1                       [1536, 512]
  Wfold  = mw_gpi[:1536] @ w1 + mw_gpe @ M          [1536, 512]
  bfold  = gpe_b @ M + gpi_b @ w1 + b1              [512]
  h1 = relu(x @ Wfold + bfold)   -> h2 -> out       (per batch row)

The fold itself is SHARDED across the 8 cores: core c computes rows
[c*192, (c+1)*192) of M (then of Wfold), which takes only the
corresponding COLUMN slices of the masks/weights (sliced host-side, a
pure layout op) — so each core streams ~5 MB of fold operands instead
of ~28 MB, and does 1/8 of the fold matmuls. Slices are assembled with
two DRAM AllGathers (M, then Wfold).

Host prep is layout/dtype only (no FLOPs): bf16 casts, transposes of
x/gpe_w/gpi_w, and column slicing.

Per-core phases (BS = 2048 batch rows):
  F0:  stream sliced mask columns + w^T columns, DVE-multiply in place.
  F1s: M_slice = sum_v mwgpiT[v, uslice]^T @ w1[v]  -> DRAM, AllGather.
  F2s: Wf_slice = sum_v mwgpiT[v, islice]^T w1[v]
                + sum_u mwgpeT[u, islice]^T M[u]    -> DRAM, AllGather.
  bias fold: tiny matmuls on gathered M + PE transpose of [1, 512] row.
  B:   per 512-row tile: h1 = relu(Wfold^T x^T), h2, out -> [6, BS] f32;
       host transposes + concats.
"""

import numpy as np
import ml_dtypes

BF = ml_dtypes.bfloat16

NCORES = 8
B = 16384
BS = B // NCORES          # 2048 rows per core
BT = 512                  # batch tile (matmul free dim)
NBT = BS // BT            # 4
D1 = 1536                 # gpe input dim (x features)
D3 = 3072                 # gpi input dim
H = 512                   # mlp hidden
A = 6                     # action dim
SL = D1 // NCORES         # 192: fold rows per core

NI = D1 // 128            # 12 i-chunks (x features)
NU = D1 // 128            # 12 u-chunks (gpe outputs)
NV = D1 // 128            # 12 v-chunks (gpi outputs)
NH = H // 128             # 4 h-chunks (mlp hidden)

_CACHE = {}


def _build():
    import concourse.bacc as bacc
    import concourse.tile as tile
    from concourse import mybir
    from concourse.masks import make_identity

    FP32 = mybir.dt.float32
    BF16 = mybir.dt.bfloat16
    Act = mybir.ActivationFunctionType

    nc = bacc.Bacc(None, num_devices=NCORES)

    xT_d = nc.dram_tensor("xT", [D1, BS], BF16, kind="ExternalInput")
    # column slices for this core's fold rows: gpi gets [islice | uslice]
    # (384 cols), gpe gets [islice] (192 cols)
    gpims_d = nc.dram_tensor("gpims", [D1, 2 * SL], BF16, kind="ExternalInput")
    gpiwTs_d = nc.dram_tensor("gpiwTs", [D1, 2 * SL], BF16, kind="ExternalInput")
    gpems_d = nc.dram_tensor("gpems", [D1, SL], BF16, kind="ExternalInput")
    gpewTs_d = nc.dram_tensor("gpewTs", [D1, SL], BF16, kind="ExternalInput")
    w1_d = nc.dram_tensor("w1", [D1, H], BF16, kind="ExternalInput")
    w2_d = nc.dram_tensor("w2", [H, H], BF16, kind="ExternalInput")
    w3_d = nc.dram_tensor("w3", [H, A], BF16, kind="ExternalInput")
    gpeb_d = nc.dram_tensor("gpe_b", [D1], FP32, kind="ExternalInput")
    gpib_d = nc.dram_tensor("gpi_b", [D1], FP32, kind="ExternalInput")
    b1_d = nc.dram_tensor("b1", [H], FP32, kind="ExternalInput")
    b2_d = nc.dram_tensor("b2", [H], FP32, kind="ExternalInput")
    b3_d = nc.dram_tensor("b3", [A], FP32, kind="ExternalInput")
    o_d = nc.dram_tensor("out", [A, BS], FP32, kind="ExternalOutput")

    RG = [list(range(NCORES))]

    with tile.TileContext(nc) as tc:
        with (
            tc.tile_pool(name="wp", bufs=1) as wp,           # persistent
            tc.tile_pool(name="tp", bufs=2) as tp,           # wT transients
            tc.tile_pool(name="xp", bufs=3) as xp,           # x tiles
            tc.tile_pool(name="ap", bufs=1) as ap,           # activations
            tc.tile_pool(name="dp", bufs=1, space="DRAM") as dp,
            tc.tile_pool(name="psp", bufs=3, space="PSUM") as psp,
            tc.tile_pool(name="ps2", bufs=1, space="PSUM") as ps2p,
            tc.tile_pool(name="pso", bufs=2, space="PSUM") as psop,
            tc.tile_pool(name="pst", bufs=1, space="PSUM") as pstp,
        ):
            # ---- w1 first (gates F1s), then sliced gpi, gpe
            w1t = []
            for v in range(NV):
                t = wp.tile([128, H], BF16, tag=f"w1_{v}")
                nc.sync.dma_start(out=t[:, :], in_=w1_d[v * 128:(v + 1) * 128, :])
                w1t.append(t)

            # masked gpi columns, [v-part, 384]: cols 0:192 = islice,
            # 192:384 = uslice
            mwgpi = []
            for v in range(NV):
                m = wp.tile([128, 2 * SL], BF16, tag=f"mwgpi{v}")
                nc.sync.dma_start(out=m[:, :], in_=gpims_d[v * 128:(v + 1) * 128, :])
                wt = tp.tile([128, 2 * SL], BF16, tag="gwT")
                nc.gpsimd.dma_start(out=wt[:, :],
                                    in_=gpiwTs_d[v * 128:(v + 1) * 128, :])
                nc.vector.tensor_mul(m[:, :], m[:, :], wt[:, :])
                mwgpi.append(m)

            # masked gpe columns, [u-part, 192]: cols = islice
            mwgpe = []
            for u in range(NU):
                m = wp.tile([128, SL], BF16, tag=f"mwgpe{u}")
                nc.sync.dma_start(out=m[:, :], in_=gpems_d[u * 128:(u + 1) * 128, :])
                wt = tp.tile([128, SL], BF16, tag="ewT")
                nc.gpsimd.dma_start(out=wt[:, :],
                                    in_=gpewTs_d[u * 128:(u + 1) * 128, :])
                nc.vector.tensor_mul(m[:, :], m[:, :], wt[:, :])
                mwgpe.append(m)

            # ---- small stuff: w2, w3, biases
            w2t = []
            for k in range(NH):
                t = wp.tile([128, H], BF16, tag=f"w2_{k}")
                nc.sync.dma_start(out=t[:, :], in_=w2_d[k * 128:(k + 1) * 128, :])
                w2t.append(t)
            w3t = []
            for k in range(NH):
                t = wp.tile([128, A], BF16, tag=f"w3_{k}")
                nc.sync.dma_start(out=t[:, :], in_=w3_d[k * 128:(k + 1) * 128, :])
                w3t.append(t)

            ident = wp.tile([128, 128], FP32, tag="ident")
            make_identity(nc, ident[:, :])

            def load_bias_cols(b_dram, n, tag):
                nat = wp.tile([n, 128], FP32, tag=f"{tag}_nat")
                nc.sync.dma_start(out=nat[:, :],
                                  in_=b_dram.rearrange("(c p) -> c p", p=128))
                ps = pstp.tile([128, n], FP32, tag="pst")
                nc.tensor.transpose(ps[:, :], nat[:, :], ident[0:n, 0:n])
                sb = wp.tile([128, n], FP32, tag=tag)
                nc.vector.tensor_copy(sb[:, :], ps[:, :])
                return sb

            gpeb_sb = load_bias_cols(gpeb_d, NU, "gpeb")
            gpib_sb = load_bias_cols(gpib_d, NV, "gpib")
            b2_sb = load_bias_cols(b2_d, NH, "b2sb")
            gpeb_bf = wp.tile([128, NU], BF16, tag="gpebf")
            nc.vector.tensor_copy(gpeb_bf[:, :], gpeb_sb[:, :])
            gpib_bf = wp.tile([128, NV], BF16, tag="gpibf")
            nc.vector.tensor_copy(gpib_bf[:, :], gpib_sb[:, :])
            b1row = wp.tile([1, H], FP32, tag="b1row")
            nc.sync.dma_start(out=b1row[:, :],
                              in_=b1_d.rearrange("(one h) -> one h", one=1))
            b3_sb = wp.tile([A, 1], FP32, tag="b3sb")
            nc.sync.dma_start(out=b3_sb[:, :],
                              in_=b3_d.rearrange("(a one) -> a one", one=1))

            # ---- x tiles stream in the background
            xt = [[None] * NI for _ in range(NBT)]
            for t_i in range(NBT):
                for i in range(NI):
                    t = xp.tile([128, BT], BF16, tag=f"xt{i}")
                    q = nc.gpsimd if (i % 2) else nc.sync
                    q.dma_start(out=t[:, :],
                                in_=xT_d[i * 128:(i + 1) * 128,
                                         t_i * BT:(t_i + 1) * BT])
                    xt[t_i][i] = t

            # ---- F1s: M_slice[r, h] = sum_v mwgpiT[v, 1536+uslice][r] w1[v]
            # slice rows split as 128 + 64
            msl_dram = dp.tile([SL, H], BF16, tag="msl_d")
            for g, (r0, rn) in enumerate([(0, 128), (128, SL - 128)]):
                ps = psp.tile([128, H], FP32, tag="ps")
                for v in range(NV):
                    nc.tensor.matmul(ps[0:rn, :],
                                     mwgpi[v][:, SL + r0:SL + r0 + rn],
                                     w1t[v][:, :],
                                     start=(v == 0), stop=(v == NV - 1))
                sb = wp.tile([128, H], BF16, tag=f"mslice{g}")
                nc.scalar.activation(sb[0:rn, :], ps[0:rn, :], Act.Copy)
                nc.sync.dma_start(out=msl_dram[r0:r0 + rn, :], in_=sb[0:rn, :])
            mfull_dram = dp.tile([D1, H], BF16, tag="mfull_d")
            nc.gpsimd.collective_compute(
                "AllGather", mybir.AluOpType.bypass, replica_groups=RG,
                ins=[msl_dram[:, :].opt()], outs=[mfull_dram[:, :].opt()])
            Mt = []
            for u in range(NU):
                t = wp.tile([128, H], BF16, tag=f"M{u}")
                nc.sync.dma_start(out=t[:, :],
                                  in_=mfull_dram[u * 128:(u + 1) * 128, :])
                Mt.append(t)

            # ---- F2s: Wf_slice = gpi-x-part + mwgpe-slice^T @ M
            wfs_dram = dp.tile([SL, H], BF16, tag="wfs_d")
            for g, (r0, rn) in enumerate([(0, 128), (128, SL - 128)]):
                ps = psp.tile([128, H], FP32, tag="ps")
                for v in range(NV):
                    nc.tensor.matmul(ps[0:rn, :],
                                     mwgpi[v][:, r0:r0 + rn],
                                     w1t[v][:, :],
                                     start=(v == 0), stop=False)
                for u in range(NU):
                    nc.tensor.matmul(ps[0:rn, :],
                                     mwgpe[u][:, r0:r0 + rn],
                                     Mt[u][:, :],
                                     start=False, stop=(u == NU - 1))
                sb = wp.tile([128, H], BF16, tag=f"wfslice{g}")
                nc.scalar.activation(sb[0:rn, :], ps[0:rn, :], Act.Copy)
                nc.sync.dma_start(out=wfs_dram[r0:r0 + rn, :], in_=sb[0:rn, :])
            wff_dram = dp.tile([D1, H], BF16, tag="wff_d")
            nc.gpsimd.collective_compute(
                "AllGather", mybir.AluOpType.bypass, replica_groups=RG,
                ins=[wfs_dram[:, :].opt()], outs=[wff_dram[:, :].opt()])
            Wf = []
            for i in range(NI):
                t = wp.tile([128, H], BF16, tag=f"Wf{i}")
                nc.sync.dma_start(out=t[:, :],
                                  in_=wff_dram[i * 128:(i + 1) * 128, :])
                Wf.append(t)

            # ---- bias fold: bfold = gpe_b @ M + gpi_b @ w1 + b1 -> [128, 4]
            psb = ps2p.tile([1, H], FP32, tag="psb")
            for v in range(NV):
                nc.tensor.matmul(psb[:, :], gpib_bf[:, v:v + 1], w1t[v][:, :],
                                 start=(v == 0), stop=False)
            for u in range(NU):
                nc.tensor.matmul(psb[:, :], gpeb_bf[:, u:u + 1], Mt[u][:, :],
                                 start=False, stop=(u == NU - 1))
            brow = wp.tile([1, H], FP32, tag="brow")
            nc.vector.tensor_add(brow[:, :], psb[:, :], b1row[:, :])
            bfold = wp.tile([128, NH], FP32, tag="bfold")
            for c in range(NH):
                ps = pstp.tile([128, 1], FP32, tag="pstc")
                nc.tensor.transpose(ps[:, :], brow[0:1, c * 128:(c + 1) * 128],
                                    ident[0:1, 0:1])
                nc.scalar.activation(bfold[:, c:c + 1], ps[:, :], Act.Copy)

            # ---- B: batch pass over 4 tiles of 512 rows
            for t_i in range(NBT):
                h1 = []
                for hc in range(NH):
                    ps = psp.tile([128, BT], FP32, tag="ps")
                    for i in range(NI):
                        nc.tensor.matmul(ps[:, :],
                                         Wf[i][:, hc * 128:(hc + 1) * 128],
                                         xt[t_i][i][:, :],
                                         start=(i == 0), stop=(i == NI - 1))
                    h = ap.tile([128, BT], BF16, tag=f"h1_{hc}")
                    nc.scalar.activation(h[:, :], ps[:, :], Act.Relu,
                                         bias=bfold[:, hc:hc + 1])
                    h1.append(h)

                h2 = []
                for mc in range(NH):
                    ps = psp.tile([128, BT], FP32, tag="ps")
                    for k in range(NH):
                        nc.tensor.matmul(ps[:, :],
                                         w2t[k][:, mc * 128:(mc + 1) * 128],
                                         h1[k][:, :],
                                         start=(k == 0), stop=(k == NH - 1))
                    h = ap.tile([128, BT], BF16, tag=f"h2_{mc}")
                    nc.scalar.activation(h[:, :], ps[:, :], Act.Relu,
                                         bias=b2_sb[:, mc:mc + 1])
                    h2.append(h)

                pso = psop.tile([A, BT], FP32, tag="pso")
                for k in range(NH):
                    nc.tensor.matmul(pso[:, :], w3t[k][:, :], h2[k][:, :],
                                     start=(k == 0), stop=(k == NH - 1))
                osb = ap.tile([A, BT], FP32, tag="osb")
                nc.scalar.activation(osb[:, :], pso[:, :], Act.Relu,
                                     bias=b3_sb[:, 0:1])
                nc.sync.dma_start(out=o_d[:, t_i * BT:(t_i + 1) * BT],
                                  in_=osb[:, :])

    nc.finalize()
    return nc


def _get_nc():
    if "nc" not in _CACHE:
        _CACHE["nc"] = _build()
    return _CACHE["nc"]


def _prep_inputs(inputs):
    """Host-side layout/dtype prep only (no network FLOPs): bf16 casts,
    transposes, and per-core column slicing of the fold operands."""
    f = {k: np.asarray(v) for k, v in inputs.items()}
    xT = np.ascontiguousarray(f["x"].astype(BF).T)            # [1536, B]
    gpem = f["gpe_mask"].astype(BF)                           # [u, i]
    gpewT = np.ascontiguousarray(f["gpe_w"].astype(BF).T)     # [u, i]
    gpim = f["gpi_mask"].astype(BF)                           # [v, j]
    gpiwT = np.ascontiguousarray(f["gpi_w"].astype(BF).T)     # [v, j]
    shared = {
        "w1": np.ascontiguousarray(f["w1"].astype(BF)),
        "w2": np.ascontiguousarray(f["w2"].astype(BF)),
        "w3": np.ascontiguousarray(f["w3"].astype(BF)),
        "gpe_b": np.ascontiguousarray(f["gpe_b"], dtype=np.float32),
        "gpi_b": np.ascontiguousarray(f["gpi_b"], dtype=np.float32),
        "b1": np.ascontiguousarray(f["b1"], dtype=np.float32),
        "b2": np.ascontiguousarray(f["b2"], dtype=np.float32),
        "b3": np.ascontiguousarray(f["b3"], dtype=np.float32),
    }
    in_maps = []
    for c in range(NCORES):
        isl = slice(c * SL, (c + 1) * SL)
        usl = slice(D1 + c * SL, D1 + (c + 1) * SL)
        in_maps.append(dict(
            shared,
            xT=np.ascontiguousarray(xT[:, c * BS:(c + 1) * BS]),
            gpims=np.ascontiguousarray(
                np.concatenate([gpim[:, isl], gpim[:, usl]], axis=1)),
            gpiwTs=np.ascontiguousarray(
                np.concatenate([gpiwT[:, isl], gpiwT[:, usl]], axis=1)),
            gpems=np.ascontiguousarray(gpem[:, isl]),
            gpewTs=np.ascontiguousarray(gpewT[:, isl]),
        ))
    return in_maps


def _run(inputs, trace=False):
    from concourse.bass_utils import run_bass_kernel_spmd

    nc = _get_nc()
    in_maps = _prep_inputs(inputs)
    res = run_bass_kernel_spmd(nc, in_maps, list(range(NCORES)), trace=trace)
    out = np.concatenate(
        [np.asarray(res.results[c]["out"]).T for c in range(NCORES)], axis=0)
    return out.astype(np.float32), res


def kernel(**inputs):
    out, _ = _run(inputs, trace=False)
    return out



# revision 2
# speedup vs baseline: 1.0182x; 1.0182x over previous
"""CTBG circuit kernel for Trainium2, data-parallel over batch on 8 NeuronCores.

Network (per reference):
  gpe_out = x @ (gpe_w * gpe_mask.T) + gpe_b              [B, 1536]
  gpi_in  = concat([x, gpe_out], -1)                      [B, 3072]
  gpi_out = gpi_in @ (gpi_w * gpi_mask.T) + gpi_b         [B, 3072] @ [3072, 1536]
  h1 = relu(gpi_out @ w1 + b1); h2 = relu(h1 @ w2 + b2)
  out = relu(h2 @ w3 + b3)                                [B, 6]

Key algebraic identity: gpe_out and gpi_out feed forward with no
intervening nonlinearity, so the masked front end folds into one
[1536, 512] weight computed ON DEVICE once per launch:

  A  = gpe_w * gpe_mask.T          [1536 i, 1536 u]
  Bx = (gpi_w * gpi_mask.T)[:1536] [1536 i, 1536 v]
  Bu = (gpi_w * gpi_mask.T)[1536:] [1536 u, 1536 v]
  M_u   = Bu @ w1                  [1536, 512]
  Wfold = Bx @ w1 + A @ M_u        [1536, 512]
  bfold = gpe_b @ M_u + gpi_b @ w1 + b1
  h1 = relu(x @ Wfold + bfold) -> h2 -> out   (per batch row)

The fold is SHARDED: core c computes rows [c*192, (c+1)*192) of M_u
(then of Wfold) from the corresponding COLUMN slices of the transposed
masks/weights (sliced host-side, pure layout), then two DRAM
AllGathers (Shared outputs) assemble the full M_u / Wfold.

Scheduling (the point of this version vs the first):
 - AllGather outputs are addr_space="Shared" (HBM-HBM fast path).
 - gpsimd queue carries ONLY the two collectives, so the triggers
   fire immediately (no x-DMA head-of-line blocking).
 - sync queue = dependency-laden critical loads (fold operands half,
   M / Wfold reloads, fold stores); scalar queue = free-flowing
   streams (other half, masks, x tiles); fold PSUM drains on vector.
 - F2s is split: the Bx@w1 half runs DURING AllGather 1; only the
   A@M_u half waits for the gathered M.
 - The bias fold (tiny matmuls + a DRAM-roundtrip transpose of the
   [1,512] row) runs DURING AllGather 2.
 - Batch pass keeps the stationary operand fixed across the 4 batch
   tiles (i-outer, t-inner) to amortize LDWEIGHTS, 4 PSUM banks wide.
 - The per-core 192 fold rows are split 96/96 interleaved (packed
   column-permuted host-side) so each drain is one [96, 1024] tile =
   one DMA store, and out-partitions are always fully used.

Host prep is layout/dtype only (no FLOPs): bf16 casts, transposes of
x/gpe_w/gpi_w, column slicing and column permutation.
"""

import numpy as np
import ml_dtypes

BF = ml_dtypes.bfloat16

NCORES = 8
B = 16384
BS = B // NCORES          # 2048 rows per core
BT = 512                  # batch tile (matmul free dim)
NBT = BS // BT            # 4
D1 = 1536                 # gpe input dim (x features)
H = 512                   # mlp hidden
A = 6                     # action dim
SL = D1 // NCORES         # 192: fold rows per core
HSL = SL // 2             # 96: interleaved half-slice

NI = D1 // 128            # 12 i-chunks (x features)
NU = D1 // 128            # 12 u-chunks (gpe outputs)
NV = D1 // 128            # 12 v-chunks (gpi outputs)
NH = H // 128             # 4 h-chunks (mlp hidden)

_CACHE = {}


def _build():
    import concourse.bacc as bacc
    import concourse.tile as tile
    from concourse import mybir

    FP32 = mybir.dt.float32
    BF16 = mybir.dt.bfloat16
    Act = mybir.ActivationFunctionType

    nc = bacc.Bacc(None, num_devices=NCORES)

    xT_d = nc.dram_tensor("xT", [D1, BS], BF16, kind="ExternalInput")
    # packed per-core fold operands, each [1536, 384] = [masked-cols | wT-cols]
    # with the 192 slice columns permuted even-first (see host prep)
    gpiu_d = nc.dram_tensor("gpiu", [D1, 2 * SL], BF16, kind="ExternalInput")
    gpii_d = nc.dram_tensor("gpii", [D1, 2 * SL], BF16, kind="ExternalInput")
    gpep_d = nc.dram_tensor("gpep", [D1, 2 * SL], BF16, kind="ExternalInput")
    w1_d = nc.dram_tensor("w1", [D1, H], BF16, kind="ExternalInput")
    w2_d = nc.dram_tensor("w2", [H, H], BF16, kind="ExternalInput")
    w3_d = nc.dram_tensor("w3", [H, A], BF16, kind="ExternalInput")
    gpeb_d = nc.dram_tensor("gpe_b", [D1], FP32, kind="ExternalInput")
    gpib_d = nc.dram_tensor("gpi_b", [D1], FP32, kind="ExternalInput")
    b1_d = nc.dram_tensor("b1", [H], FP32, kind="ExternalInput")
    b2_d = nc.dram_tensor("b2", [H], FP32, kind="ExternalInput")
    b3_d = nc.dram_tensor("b3", [A], FP32, kind="ExternalInput")
    o_d = nc.dram_tensor("out", [A, BS], FP32, kind="ExternalOutput")

    RG = [list(range(NCORES))]

    with tile.TileContext(nc) as tc:
        with (
            tc.tile_pool(name="wp", bufs=1) as wp,           # persistent
            tc.tile_pool(name="ap", bufs=1) as ap,           # activations
            tc.tile_pool(name="dp", bufs=1, space="DRAM") as dp,
            tc.tile_pool(name="psp", bufs=8, space="PSUM") as psp,
        ):
            def ps_tile():
                return psp.tile([128, BT], FP32, tag="ps", name="ps")

            # ---- fold operand stream: gpi uslice + w1, alternating queues
            gpiu = []
            w1t = []
            for v in range(NV):
                q = nc.sync if (v % 2 == 0) else nc.scalar
                m = wp.tile([128, 2 * SL], BF16, tag=f"gpiu{v}")
                q.dma_start(out=m[:, :], in_=gpiu_d[v * 128:(v + 1) * 128, :])
                w = wp.tile([128, H], BF16, tag=f"w1_{v}")
                q.dma_start(out=w[:, :], in_=w1_d[v * 128:(v + 1) * 128, :])
                nc.vector.tensor_mul(m[:, 0:SL], m[:, 0:SL], m[:, SL:2 * SL])
                gpiu.append(m)
                w1t.append(w)

            # ---- F1s: M_u slice, two interleaved 96-row groups
            ps_m = [ps_tile() for _ in range(2)]
            for v in range(NV):
                for g in range(2):
                    nc.tensor.matmul(ps_m[g][0:HSL, :],
                                     gpiu[v][:, g * HSL:(g + 1) * HSL],
                                     w1t[v][:, :],
                                     start=(v == 0), stop=(v == NV - 1))
            msb = wp.tile([HSL, 2 * H], BF16, tag="msb")
            for g in range(2):
                nc.vector.tensor_copy(msb[:, g * H:(g + 1) * H],
                                      ps_m[g][0:HSL, :])
            msl_dram = dp.tile([HSL, 2 * H], BF16, tag="msl_d")
            nc.sync.dma_start(out=msl_dram[:, :], in_=msb[:, :])
            mfull_dram = dp.tile([D1, H], BF16, tag="mfull_d",
                                 addr_space="Shared")
            nc.gpsimd.collective_compute(
                "AllGather", mybir.AluOpType.bypass, replica_groups=RG,
                ins=[msl_dram[:, :].opt()], outs=[mfull_dram[:, :].opt()])

            # ---- streams that overlap AG1: gpi islice, gpe islice, biases,
            # w2/w3, then x tiles (all on scalar; sync is reserved for the
            # AG1-dependent M reloads)
            gpii = []
            for v in range(NV):
                m = wp.tile([128, 2 * SL], BF16, tag=f"gpii{v}")
                nc.scalar.dma_start(out=m[:, :],
                                    in_=gpii_d[v * 128:(v + 1) * 128, :])
                nc.vector.tensor_mul(m[:, 0:SL], m[:, 0:SL], m[:, SL:2 * SL])
                gpii.append(m)
            gpep = []
            for u in range(NU):
                m = wp.tile([128, 2 * SL], BF16, tag=f"gpep{u}")
                nc.scalar.dma_start(out=m[:, :],
                                    in_=gpep_d[u * 128:(u + 1) * 128, :])
                nc.vector.tensor_mul(m[:, 0:SL], m[:, 0:SL], m[:, SL:2 * SL])
                gpep.append(m)

            # bias columns loaded directly transposed via strided DRAM APs
            gpeb_sb = wp.tile([128, NU], FP32, tag="gpeb")
            nc.scalar.dma_start(out=gpeb_sb[:, :],
                                in_=gpeb_d.rearrange("(c p) -> p c", p=128))
            gpib_sb = wp.tile([128, NV], FP32, tag="gpib")
            nc.scalar.dma_start(out=gpib_sb[:, :],
                                in_=gpib_d.rearrange("(c p) -> p c", p=128))
            b2_sb = wp.tile([128, NH], FP32, tag="b2sb")
            nc.scalar.dma_start(out=b2_sb[:, :],
                                in_=b2_d.rearrange("(c p) -> p c", p=128))
            b3_sb = wp.tile([A, 1], FP32, tag="b3sb")
            nc.scalar.dma_start(out=b3_sb[:, :],
                                in_=b3_d.rearrange("(a one) -> a one", one=1))
            b1row = wp.tile([1, H], FP32, tag="b1row")
            nc.scalar.dma_start(out=b1row[:, :],
                                in_=b1_d.rearrange("(one h) -> one h", one=1))
            gpeb_bf = wp.tile([128, NU], BF16, tag="gpebf")
            nc.vector.tensor_copy(gpeb_bf[:, :], gpeb_sb[:, :])
            gpib_bf = wp.tile([128, NV], BF16, tag="gpibf")
            nc.vector.tensor_copy(gpib_bf[:, :], gpib_sb[:, :])

            w2t = []
            for k in range(NH):
                t = wp.tile([128, H], BF16, tag=f"w2_{k}")
                nc.scalar.dma_start(out=t[:, :], in_=w2_d[k * 128:(k + 1) * 128, :])
                w2t.append(t)
            w3t = []
            for k in range(NH):
                t = wp.tile([128, A], BF16, tag=f"w3_{k}")
                nc.scalar.dma_start(out=t[:, :], in_=w3_d[k * 128:(k + 1) * 128, :])
                w3t.append(t)

            # ---- F2s part a (no M dependency): Wf_slice += Bx @ w1
            ps_wf = [ps_tile() for _ in range(2)]
            for v in range(NV):
                for g in range(2):
                    nc.tensor.matmul(ps_wf[g][0:HSL, :],
                                     gpii[v][:, g * HSL:(g + 1) * HSL],
                                     w1t[v][:, :],
                                     start=(v == 0), stop=False)

            # ---- M reloads (block sync queue until AG1 completes)
            Mt = []
            for u in range(NU):
                t = wp.tile([128, H], BF16, tag=f"M{u}")
                nc.sync.dma_start(out=t[:, :],
                                  in_=mfull_dram[u * 128:(u + 1) * 128, :])
                Mt.append(t)

            # ---- x tiles stream on scalar behind the small stuff
            xt = [[None] * NI for _ in range(NBT)]
            for t_i in range(NBT):
                for i in range(NI):
                    t = wp.tile([128, BT], BF16, tag=f"x{t_i}_{i}")
                    nc.scalar.dma_start(out=t[:, :],
                                        in_=xT_d[i * 128:(i + 1) * 128,
                                                 t_i * BT:(t_i + 1) * BT])
                    xt[t_i][i] = t

            # ---- F2s part b: Wf_slice += A @ M_u
            for u in range(NU):
                for g in range(2):
                    nc.tensor.matmul(ps_wf[g][0:HSL, :],
                                     gpep[u][:, g * HSL:(g + 1) * HSL],
                                     Mt[u][:, :],
                                     start=False, stop=(u == NU - 1))
            wsb = wp.tile([HSL, 2 * H], BF16, tag="wsb")
            for g in range(2):
                nc.vector.tensor_copy(wsb[:, g * H:(g + 1) * H],
                                      ps_wf[g][0:HSL, :])
            wfs_dram = dp.tile([HSL, 2 * H], BF16, tag="wfs_d")
            nc.sync.dma_start(out=wfs_dram[:, :], in_=wsb[:, :])
            wff_dram = dp.tile([D1, H], BF16, tag="wff_d",
                               addr_space="Shared")
            nc.gpsimd.collective_compute(
                "AllGather", mybir.AluOpType.bypass, replica_groups=RG,
                ins=[wfs_dram[:, :].opt()], outs=[wff_dram[:, :].opt()])

            # ---- bias fold during AG2: bfold = gpe_b @ M + gpi_b @ w1 + b1
            psb = ps_tile()
            for v in range(NV):
                nc.tensor.matmul(psb[0:1, :], gpib_bf[:, v:v + 1], w1t[v][:, :],
                                 start=(v == 0), stop=False)
            for u in range(NU):
                nc.tensor.matmul(psb[0:1, :], gpeb_bf[:, u:u + 1], Mt[u][:, :],
                                 start=False, stop=(u == NU - 1))
            brow = wp.tile([1, H], FP32, tag="brow")
            nc.vector.tensor_add(brow[:, :], psb[0:1, :], b1row[:, :])
            # transpose [1, 512] -> [128, 4] via a DRAM roundtrip (PE stays free)
            brow_dram = dp.tile([1, H], FP32, tag="brow_d")
            nc.sync.dma_start(out=brow_dram[:, :], in_=brow[:, :])
            bfold = wp.tile([128, NH], FP32, tag="bfold")
            nc.sync.dma_start(
                out=bfold[:, :],
                in_=brow_dram[:, :].rearrange("one (c p) -> p (one c)", p=128))

            # ---- Wfold reloads (block sync queue until AG2 completes)
            Wf = []
            for i in range(NI):
                t = wp.tile([128, H], BF16, tag=f"Wf{i}")
                nc.sync.dma_start(out=t[:, :],
                                  in_=wff_dram[i * 128:(i + 1) * 128, :])
                Wf.append(t)

            # ---- batch pass: 4 tiles of 512 rows, stationary reused across
            # tiles (i-outer, t-inner), 4 PSUM banks wide
            h1 = [[None] * NH for _ in range(NBT)]
            for hc in range(NH):
                ps1 = [ps_tile() for _ in range(NBT)]
                for i in range(NI):
                    for t_i in range(NBT):
                        nc.tensor.matmul(ps1[t_i][:, :],
                                         Wf[i][:, hc * 128:(hc + 1) * 128],
                                         xt[t_i][i][:, :],
                                         start=(i == 0), stop=(i == NI - 1))
                for t_i in range(NBT):
                    h = ap.tile([128, BT], BF16, tag=f"h1_{t_i}_{hc}")
                    nc.scalar.activation(h[:, :], ps1[t_i][:, :], Act.Relu,
                                         bias=bfold[:, hc:hc + 1])
                    h1[t_i][hc] = h

            h2 = [[None] * NH for _ in range(NBT)]
            for mc in range(NH):
                ps2 = [ps_tile() for _ in range(NBT)]
                for k in range(NH):
                    for t_i in range(NBT):
                        nc.tensor.matmul(ps2[t_i][:, :],
                                         w2t[k][:, mc * 128:(mc + 1) * 128],
                                         h1[t_i][k][:, :],
                                         start=(k == 0), stop=(k == NH - 1))
                for t_i in range(NBT):
                    h = ap.tile([128, BT], BF16, tag=f"h2_{t_i}_{mc}")
                    nc.scalar.activation(h[:, :], ps2[t_i][:, :], Act.Relu,
                                         bias=b2_sb[:, mc:mc + 1])
                    h2[t_i][mc] = h

            pso = [ps_tile() for _ in range(NBT)]
            for k in range(NH):
                for t_i in range(NBT):
                    nc.tensor.matmul(pso[t_i][0:A, :], w3t[k][:, :],
                                     h2[t_i][k][:, :],
                                     start=(k == 0), stop=(k == NH - 1))
            for t_i in range(NBT):
                osb = ap.tile([A, BT], FP32, tag=f"osb{t_i}")
                nc.scalar.activation(osb[:, :], pso[t_i][0:A, :], Act.Relu,
                                     bias=b3_sb[:, 0:1])
                nc.sync.dma_start(out=o_d[:, t_i * BT:(t_i + 1) * BT],
                                  in_=osb[:, :])

    nc.finalize()
    return nc


def _get_nc():
    if "nc" not in _CACHE:
        _CACHE["nc"] = _build()
    return _CACHE["nc"]


def _prep_inputs(inputs):
    """Host-side layout/dtype prep only (no network FLOPs): bf16 casts,
    transposes, per-core column slicing and column permutation of the
    fold operands."""
    f = {k: np.asarray(v) for k, v in inputs.items()}
    xT = np.ascontiguousarray(f["x"].astype(BF).T)            # [1536, B]
    gpem = f["gpe_mask"].astype(BF)                           # [u, i]
    gpewT = np.ascontiguousarray(f["gpe_w"].astype(BF).T)     # [u, i]
    gpim = f["gpi_mask"].astype(BF)                           # [v, j]
    gpiwT = np.ascontiguousarray(f["gpi_w"].astype(BF).T)     # [v, j]
    # packed position g*96 + p  <->  logical slice column 2p + g
    perm = np.concatenate([np.arange(0, SL, 2), np.arange(1, SL, 2)])
    shared = {
        "w1": np.ascontiguousarray(f["w1"].astype(BF)),
        "w2": np.ascontiguousarray(f["w2"].astype(BF)),
        "w3": np.ascontiguousarray(f["w3"].astype(BF)),
        "gpe_b": np.ascontiguousarray(f["gpe_b"], dtype=np.float32),
        "gpi_b": np.ascontiguousarray(f["gpi_b"], dtype=np.float32),
        "b1": np.ascontiguousarray(f["b1"], dtype=np.float32),
        "b2": np.ascontiguousarray(f["b2"], dtype=np.float32),
        "b3": np.ascontiguousarray(f["b3"], dtype=np.float32),
    }
    in_maps = []
    for c in range(NCORES):
        isl = np.arange(c * SL, (c + 1) * SL)[perm]
        usl = D1 + isl
        in_maps.append(dict(
            shared,
            xT=np.ascontiguousarray(xT[:, c * BS:(c + 1) * BS]),
            gpiu=np.ascontiguousarray(
                np.concatenate([gpim[:, usl], gpiwT[:, usl]], axis=1)),
            gpii=np.ascontiguousarray(
                np.concatenate([gpim[:, isl], gpiwT[:, isl]], axis=1)),
            gpep=np.ascontiguousarray(
                np.concatenate([gpem[:, isl], gpewT[:, isl]], axis=1)),
        ))
    return in_maps


def _run(inputs, trace=False):
    from concourse.bass_utils import run_bass_kernel_spmd

    nc = _get_nc()
    in_maps = _prep_inputs(inputs)
    res = run_bass_kernel_spmd(nc, in_maps, list(range(NCORES)), trace=trace)
    out = np.concatenate(
        [np.asarray(res.results[c]["out"]).T for c in range(NCORES)], axis=0)
    return out.astype(np.float32), res


def kernel(**inputs):
    out, _ = _run(inputs, trace=False)
    return out


# revision 4
# speedup vs baseline: 1.0631x; 1.0441x over previous
"""CTBG circuit kernel for Trainium2, data-parallel over batch on 8 NeuronCores.

Network (per reference):
  gpe_out = x @ (gpe_w * gpe_mask.T) + gpe_b              [B, 1536]
  gpi_in  = concat([x, gpe_out], -1)                      [B, 3072]
  gpi_out = gpi_in @ (gpi_w * gpi_mask.T) + gpi_b         [B, 3072] @ [3072, 1536]
  h1 = relu(gpi_out @ w1 + b1); h2 = relu(h1 @ w2 + b2)
  out = relu(h2 @ w3 + b3)                                [B, 6]

Key algebraic identity: gpe_out and gpi_out feed forward with no
intervening nonlinearity, so the masked front end folds into one
[1536, 512] weight computed ON DEVICE once per launch:

  A  = gpe_w * gpe_mask.T          [1536 i, 1536 u]
  Bx = (gpi_w * gpi_mask.T)[:1536] [1536 i, 1536 v]
  Bu = (gpi_w * gpi_mask.T)[1536:] [1536 u, 1536 v]
  Wfold = Bx @ w1 + A @ (Bu @ w1)  [1536, 512]
  bfold = gpe_b @ (Bu @ w1) + gpi_b @ w1 + b1
  h1 = relu(x @ Wfold + bfold) -> h2 -> out   (per batch row)

Distribution: a fixed ~36us collectives-init barrier on this platform
gates the FIRST collective completion to ~90us into the launch, so
chained collectives (gather M, then gather Wfold) are poison.  Instead
every core computes a full-shape PARTIAL of Wfold from purely local
slices, and ONE AllReduce(add) sums them:

  core c:  M_c = Bu[usl_c] @ w1                 [192, 512]  (local)
           P_c = Bx[:, vsl_c] @ w1[vsl_c]       [1536, 512] (partial
               + A[:, usl_c] @ M_c                            sums)
           prow_c = gpe_b[usl_c] @ M_c + gpi_b[vsl_c] @ w1[vsl_c]
  AllReduce over cores: Wfold = sum_c P_c ; bias row = sum_c prow_c.

The AllReduce is split into two h-halves so the batch pass starts on
h-columns 0:256 while the second half is still on the wire.  The
batch pass keeps the stationary operand fixed across the 4 batch
tiles (i-outer, t-inner) to amortize LDWEIGHTS, 4 PSUM banks wide.
gpsimd queue carries only collectives; sync carries the
dependency-laden loads; scalar carries free-flowing streams (x).

Host prep is layout/dtype only (no FLOPs): bf16 casts, transposes of
x/gpe_w/gpi_w, row/column slicing and an even/odd interleave
permutation of each 192-row slice (so the two 96-row PE groups are
contiguous and drains are single DMAs).
"""

import numpy as np
import ml_dtypes

BF = ml_dtypes.bfloat16

NCORES = 8
B = 16384
BS = B // NCORES          # 2048 rows per core
BT = 512                  # batch tile (matmul free dim)
NBT = BS // BT            # 4
D1 = 1536                 # gpe input dim (x features)
H = 512                   # mlp hidden
HH = H // 2               # 256: AllReduce column half
A = 6                     # action dim
SL = D1 // NCORES         # 192: fold rows per core
HSL = SL // 2             # 96: interleaved half-slice
PR = D1 + 1               # AllReduce rows: 1536 Wfold + 1 bias row

NI = D1 // 128            # 12 i-chunks (x features)
NV = D1 // 128            # 12 v-chunks (gpi outputs)
NH = H // 128             # 4 h-chunks (mlp hidden)

_CACHE = {}


def _build():
    import concourse.bacc as bacc
    import concourse.tile as tile
    from concourse import mybir

    FP32 = mybir.dt.float32
    BF16 = mybir.dt.bfloat16
    Act = mybir.ActivationFunctionType

    nc = bacc.Bacc(None, num_devices=NCORES)

    xT_d = nc.dram_tensor("xT", [D1, BS], BF16, kind="ExternalInput")
    # [1536, 384] = [masked uslice cols | wT uslice cols], interleave-permuted
    gpiu_d = nc.dram_tensor("gpiu", [D1, 2 * SL], BF16, kind="ExternalInput")
    # [192, 3072] = [mask | wT] rows vsl (BxT) / usl (AT), interleave-permuted
    bxp_d = nc.dram_tensor("bxp", [SL, 2 * D1], BF16, kind="ExternalInput")
    ap_d = nc.dram_tensor("apk", [SL, 2 * D1], BF16, kind="ExternalInput")
    w1_d = nc.dram_tensor("w1", [D1, H], BF16, kind="ExternalInput")
    w1vs_d = nc.dram_tensor("w1vs", [SL, H], BF16, kind="ExternalInput")
    w2_d = nc.dram_tensor("w2", [H, H], BF16, kind="ExternalInput")
    w3_d = nc.dram_tensor("w3", [H, A], BF16, kind="ExternalInput")
    gpebp_d = nc.dram_tensor("gpebp", [HSL, 2], FP32, kind="ExternalInput")
    gpibp_d = nc.dram_tensor("gpibp", [HSL, 2], FP32, kind="ExternalInput")
    b1_d = nc.dram_tensor("b1", [H], FP32, kind="ExternalInput")
    b2_d = nc.dram_tensor("b2", [H], FP32, kind="ExternalInput")
    b3_d = nc.dram_tensor("b3", [A], FP32, kind="ExternalInput")
    o_d = nc.dram_tensor("out", [A, BS], FP32, kind="ExternalOutput")

    RG = [list(range(NCORES))]

    with tile.TileContext(nc) as tc:
        with (
            tc.tile_pool(name="wp", bufs=1) as wp,           # persistent
            tc.tile_pool(name="pc", bufs=3) as pcp,          # P drain staging
            tc.tile_pool(name="ap", bufs=1) as ap,           # activations
            tc.tile_pool(name="dp", bufs=1, space="DRAM") as dp,
            tc.tile_pool(name="psp", bufs=8, space="PSUM") as psp,
        ):
            def ps_tile():
                return psp.tile([128, BT], FP32, tag="ps", name="ps")

            # ---- fold operand stream: gpi uslice + w1 (F1s), alternating
            gpiu = []
            w1t = []
            for v in range(NV):
                q = nc.sync if (v % 2 == 0) else nc.scalar
                m = wp.tile([128, 2 * SL], BF16, tag=f"gpiu{v}")
                q.dma_start(out=m[:, :], in_=gpiu_d[v * 128:(v + 1) * 128, :])
                w = wp.tile([128, H], BF16, tag=f"w1_{v}")
                q.dma_start(out=w[:, :], in_=w1_d[v * 128:(v + 1) * 128, :])
                nc.vector.tensor_mul(m[:, 0:SL], m[:, 0:SL], m[:, SL:2 * SL])
                gpiu.append(m)
                w1t.append(w)

            # ---- partial-P operands: BxT/AT row-slices, w1 row-slice
            bxp, apk, w1vs = [], [], []
            for g in range(2):
                t = wp.tile([HSL, 2 * D1], BF16, tag=f"bxp{g}")
                nc.sync.dma_start(out=t[:, :],
                                  in_=bxp_d[g * HSL:(g + 1) * HSL, :])
                nc.vector.tensor_mul(t[:, 0:D1], t[:, 0:D1], t[:, D1:2 * D1])
                bxp.append(t)
                t = wp.tile([HSL, 2 * D1], BF16, tag=f"apk{g}")
                nc.scalar.dma_start(out=t[:, :],
                                    in_=ap_d[g * HSL:(g + 1) * HSL, :])
                nc.vector.tensor_mul(t[:, 0:D1], t[:, 0:D1], t[:, D1:2 * D1])
                apk.append(t)
                t = wp.tile([HSL, H], BF16, tag=f"w1vs{g}")
                nc.sync.dma_start(out=t[:, :],
                                  in_=w1vs_d[g * HSL:(g + 1) * HSL, :])
                w1vs.append(t)

            # small loads
            gpebp = wp.tile([HSL, 2], FP32, tag="gpebp")
            nc.scalar.dma_start(out=gpebp[:, :], in_=gpebp_d[:, :])
            gpibp = wp.tile([HSL, 2], FP32, tag="gpibp")
            nc.scalar.dma_start(out=gpibp[:, :], in_=gpibp_d[:, :])
            gpebf = wp.tile([HSL, 2], BF16, tag="gpebf")
            nc.vector.tensor_copy(gpebf[:, :], gpebp[:, :])
            gpibf = wp.tile([HSL, 2], BF16, tag="gpibf")
            nc.vector.tensor_copy(gpibf[:, :], gpibp[:, :])
            b2_sb = wp.tile([128, NH], FP32, tag="b2sb")
            nc.scalar.dma_start(out=b2_sb[:, :],
                                in_=b2_d.rearrange("(c p) -> p c", p=128))
            b3_sb = wp.tile([A, 1], FP32, tag="b3sb")
            nc.scalar.dma_start(out=b3_sb[:, :],
                                in_=b3_d.rearrange("(a one) -> a one", one=1))
            b1row = wp.tile([1, H], FP32, tag="b1row")
            nc.scalar.dma_start(out=b1row[:, :],
                                in_=b1_d.rearrange("(one h) -> one h", one=1))
            w2t = []
            for k in range(NH):
                t = wp.tile([128, H], BF16, tag=f"w2_{k}")
                nc.scalar.dma_start(out=t[:, :], in_=w2_d[k * 128:(k + 1) * 128, :])
                w2t.append(t)
            w3t = []
            for k in range(NH):
                t = wp.tile([128, A], BF16, tag=f"w3_{k}")
                nc.scalar.dma_start(out=t[:, :], in_=w3_d[k * 128:(k + 1) * 128, :])
                w3t.append(t)

            # ---- F1s: local M slice, two interleaved 96-row groups ->
            # msb[:, g*512:(g+1)*512] holds M rows {2p+g} in bf16
            ps_m = [ps_tile() for _ in range(2)]
            for v in range(NV):
                for g in range(2):
                    nc.tensor.matmul(ps_m[g][0:HSL, :],
                                     gpiu[v][:, g * HSL:(g + 1) * HSL],
                                     w1t[v][:, :],
                                     start=(v == 0), stop=(v == NV - 1))
            msb = wp.tile([HSL, 2 * H], BF16, tag="msb")
            for g in range(2):
                nc.vector.tensor_copy(msb[:, g * H:(g + 1) * H],
                                      ps_m[g][0:HSL, :])

            # ---- partial P chunks + column-split drain to DRAM
            pa_dram = dp.tile([PR, HH], BF16, tag="pa_d")
            pb_dram = dp.tile([PR, HH], BF16, tag="pb_d")
            for i in range(NI):
                ps = ps_tile()
                for g in range(2):
                    nc.tensor.matmul(ps[:, :],
                                     bxp[g][:, i * 128:(i + 1) * 128],
                                     w1vs[g][:, :],
                                     start=(g == 0), stop=False)
                for g in range(2):
                    nc.tensor.matmul(ps[:, :],
                                     apk[g][:, i * 128:(i + 1) * 128],
                                     msb[:, g * H:(g + 1) * H],
                                     start=False, stop=(g == 1))
                sb = pcp.tile([128, BT], BF16, tag="pcs")
                nc.vector.tensor_copy(sb[:, :], ps[:, :])
                nc.sync.dma_start(out=pa_dram[i * 128:(i + 1) * 128, :],
                                  in_=sb[:, 0:HH])
                nc.sync.dma_start(out=pb_dram[i * 128:(i + 1) * 128, :],
                                  in_=sb[:, HH:2 * HH])

            # bias partial row (row 1536 of the AllReduce payload)
            psb = ps_tile()
            for g in range(2):
                nc.tensor.matmul(psb[0:1, :], gpibf[:, g:g + 1], w1vs[g][:, :],
                                 start=(g == 0), stop=False)
            for g in range(2):
                nc.tensor.matmul(psb[0:1, :], gpebf[:, g:g + 1],
                                 msb[:, g * H:(g + 1) * H],
                                 start=False, stop=(g == 1))
            prow = wp.tile([1, H], BF16, tag="prow")
            nc.vector.tensor_copy(prow[:, :], psb[0:1, :])
            nc.sync.dma_start(out=pa_dram[D1:PR, :], in_=prow[:, 0:HH])
            nc.sync.dma_start(out=pb_dram[D1:PR, :], in_=prow[:, HH:2 * HH])

            # ---- ONE AllReduce, split into two h-halves
            wfa_dram = dp.tile([PR, HH], BF16, tag="wfa_d", addr_space="Shared")
            wfb_dram = dp.tile([PR, HH], BF16, tag="wfb_d", addr_space="Shared")
            nc.gpsimd.collective_compute(
                "AllReduce", mybir.AluOpType.add, replica_groups=RG,
                ins=[pa_dram[:, :].opt()], outs=[wfa_dram[:, :].opt()])
            nc.gpsimd.collective_compute(
                "AllReduce", mybir.AluOpType.add, replica_groups=RG,
                ins=[pb_dram[:, :].opt()], outs=[wfb_dram[:, :].opt()])

            # ---- x tiles stream on scalar meanwhile
            xt = [[None] * NI for _ in range(NBT)]
            for t_i in range(NBT):
                for i in range(NI):
                    t = wp.tile([128, BT], BF16, tag=f"x{t_i}_{i}")
                    nc.scalar.dma_start(out=t[:, :],
                                        in_=xT_d[i * 128:(i + 1) * 128,
                                                 t_i * BT:(t_i + 1) * BT])
                    xt[t_i][i] = t

            # ---- Wfold reloads: a-halves (gated on AR a), then b-halves
            Wf = []
            for i in range(NI):
                t = wp.tile([128, H], BF16, tag=f"Wf{i}")
                nc.sync.dma_start(out=t[:, 0:HH],
                                  in_=wfa_dram[i * 128:(i + 1) * 128, :])
                Wf.append(t)
            browb = wp.tile([1, H], BF16, tag="browb")
            nc.sync.dma_start(out=browb[:, 0:HH], in_=wfa_dram[D1:PR, :])
            for i in range(NI):
                nc.sync.dma_start(out=Wf[i][:, HH:H],
                                  in_=wfb_dram[i * 128:(i + 1) * 128, :])
            nc.sync.dma_start(out=browb[:, HH:H], in_=wfb_dram[D1:PR, :])

            # bias row + b1, transposed [1,512] -> [128,4] via DRAM roundtrip,
            # one half per AllReduce so hc 0/1 activations don't wait on AR b.
            brow = wp.tile([1, H], FP32, tag="brow")
            brow_dram = dp.tile([1, H], FP32, tag="brow_d")
            bfold = wp.tile([128, NH], FP32, tag="bfold")

            def bias_half(half):
                lo, hi = half * HH, (half + 1) * HH
                nc.vector.tensor_add(brow[:, lo:hi], browb[:, lo:hi],
                                     b1row[:, lo:hi])
                nc.scalar.dma_start(out=brow_dram[:, lo:hi], in_=brow[:, lo:hi])
                nc.scalar.dma_start(
                    out=bfold[:, half * 2:(half + 1) * 2],
                    in_=brow_dram[:, lo:hi].rearrange(
                        "one (c p) -> p (one c)", p=128))

            bias_half(0)

            # ---- batch pass: 4 tiles of 512 rows, stationary reused across
            # tiles (i-outer, t-inner), 4 PSUM banks wide
            h1 = [[None] * NH for _ in range(NBT)]
            for hc in range(NH):
                ps1 = [ps_tile() for _ in range(NBT)]
                for i in range(NI):
                    for t_i in range(NBT):
                        nc.tensor.matmul(ps1[t_i][:, :],
                                         Wf[i][:, hc * 128:(hc + 1) * 128],
                                         xt[t_i][i][:, :],
                                         start=(i == 0), stop=(i == NI - 1))
                for t_i in range(NBT):
                    h = ap.tile([128, BT], BF16, tag=f"h1_{t_i}_{hc}")
                    nc.scalar.activation(h[:, :], ps1[t_i][:, :], Act.Relu,
                                         bias=bfold[:, hc:hc + 1])
                    h1[t_i][hc] = h
                if hc == 1:
                    bias_half(1)

            h2 = [[None] * NH for _ in range(NBT)]
            for mc in range(NH):
                ps2 = [ps_tile() for _ in range(NBT)]
                for k in range(NH):
                    for t_i in range(NBT):
                        nc.tensor.matmul(ps2[t_i][:, :],
                                         w2t[k][:, mc * 128:(mc + 1) * 128],
                                         h1[t_i][k][:, :],
                                         start=(k == 0), stop=(k == NH - 1))
                for t_i in range(NBT):
                    h = ap.tile([128, BT], BF16, tag=f"h2_{t_i}_{mc}")
                    nc.scalar.activation(h[:, :], ps2[t_i][:, :], Act.Relu,
                                         bias=b2_sb[:, mc:mc + 1])
                    h2[t_i][mc] = h

            pso = [ps_tile() for _ in range(NBT)]
            for k in range(NH):
                for t_i in range(NBT):
                    nc.tensor.matmul(pso[t_i][0:A, :], w3t[k][:, :],
                                     h2[t_i][k][:, :],
                                     start=(k == 0), stop=(k == NH - 1))
            for t_i in range(NBT):
                osb = ap.tile([A, BT], FP32, tag=f"osb{t_i}")
                nc.scalar.activation(osb[:, :], pso[t_i][0:A, :], Act.Relu,
                                     bias=b3_sb[:, 0:1])
                nc.sync.dma_start(out=o_d[:, t_i * BT:(t_i + 1) * BT],
                                  in_=osb[:, :])

    nc.finalize()
    return nc


def _get_nc():
    if "nc" not in _CACHE:
        _CACHE["nc"] = _build()
    return _CACHE["nc"]


def _prep_inputs(inputs):
    """Host-side layout/dtype prep only (no network FLOPs): bf16 casts,
    transposes, per-core row/column slicing and interleave permutation."""
    f = {k: np.asarray(v) for k, v in inputs.items()}
    xT = np.ascontiguousarray(f["x"].astype(BF).T)            # [1536, B]
    gpem = f["gpe_mask"].astype(BF)                           # [u, i]
    gpewT = np.ascontiguousarray(f["gpe_w"].astype(BF).T)     # [u, i]
    gpim = f["gpi_mask"].astype(BF)                           # [v, j]
    gpiwT = np.ascontiguousarray(f["gpi_w"].astype(BF).T)     # [v, j]
    w1 = f["w1"].astype(BF)
    gpe_b = np.asarray(f["gpe_b"], dtype=np.float32)
    gpi_b = np.asarray(f["gpi_b"], dtype=np.float32)
    # packed position g*96 + p  <->  logical slice index 2p + g
    perm = np.concatenate([np.arange(0, SL, 2), np.arange(1, SL, 2)])
    shared = {
        "w1": np.ascontiguousarray(w1),
        "w2": np.ascontiguousarray(f["w2"].astype(BF)),
        "w3": np.ascontiguousarray(f["w3"].astype(BF)),
        "b1": np.ascontiguousarray(f["b1"], dtype=np.float32),
        "b2": np.ascontiguousarray(f["b2"], dtype=np.float32),
        "b3": np.ascontiguousarray(f["b3"], dtype=np.float32),
    }
    in_maps = []
    for c in range(NCORES):
        sl = np.arange(c * SL, (c + 1) * SL)[perm]   # permuted local slice
        usl = D1 + sl                                # gpi columns for u-part
        in_maps.append(dict(
            shared,
            xT=np.ascontiguousarray(xT[:, c * BS:(c + 1) * BS]),
            gpiu=np.ascontiguousarray(
                np.concatenate([gpim[:, usl], gpiwT[:, usl]], axis=1)),
            bxp=np.ascontiguousarray(
                np.concatenate([gpim[sl][:, :D1], gpiwT[sl][:, :D1]], axis=1)),
            apk=np.ascontiguousarray(
                np.concatenate([gpem[sl], gpewT[sl]], axis=1)),
            w1vs=np.ascontiguousarray(w1[sl]),
            gpebp=np.ascontiguousarray(
                gpe_b[sl].reshape(2, HSL).T, dtype=np.float32),
            gpibp=np.ascontiguousarray(
                gpi_b[sl].reshape(2, HSL).T, dtype=np.float32),
        ))
    return in_maps


def _run(inputs, trace=False):
    from concourse.bass_utils import run_bass_kernel_spmd

    nc = _get_nc()
    in_maps = _prep_inputs(inputs)
    res = run_bass_kernel_spmd(nc, in_maps, list(range(NCORES)), trace=trace)
    out = np.concatenate(
        [np.asarray(res.results[c]["out"]).T for c in range(NCORES)], axis=0)
    return out.astype(np.float32), res


def kernel(**inputs):
    out, _ = _run(inputs, trace=False)
    return out


# revision 10
# speedup vs baseline: 1.1294x; 1.0623x over previous
"""CTBG circuit kernel for Trainium2, data-parallel over batch on 8 NeuronCores.

Network (per reference):
  gpe_out = x @ (gpe_w * gpe_mask.T) + gpe_b              [B, 1536]
  gpi_in  = concat([x, gpe_out], -1)                      [B, 3072]
  gpi_out = gpi_in @ (gpi_w * gpi_mask.T) + gpi_b         [B, 3072] @ [3072, 1536]
  h1 = relu(gpi_out @ w1 + b1); h2 = relu(h1 @ w2 + b2)
  out = relu(h2 @ w3 + b3)                                [B, 6]

Key algebraic identity: gpe_out and gpi_out feed forward with no
intervening nonlinearity, so the masked front end folds into one
[1536, 512] weight computed ON DEVICE once per launch:

  A  = gpe_w * gpe_mask.T          [1536 i, 1536 u]
  Bx = (gpi_w * gpi_mask.T)[:1536] [1536 i, 1536 v]
  Bu = (gpi_w * gpi_mask.T)[1536:] [1536 u, 1536 v]
  Wfold = Bx @ w1 + A @ (Bu @ w1)  [1536, 512]
  bfold = gpe_b @ (Bu @ w1) + gpi_b @ w1 + b1
  h1 = relu(x @ Wfold + bfold) -> h2 -> out   (per batch row)

Distribution: a fixed ~36us collectives-init barrier on this platform
gates the FIRST collective completion to ~90us into the launch, so
chained collectives (gather M, then gather Wfold) are poison.  Instead
every core computes a full-shape PARTIAL of Wfold from purely local
slices, and ONE AllReduce(add) sums them:

  core c:  M_c = Bu[usl_c] @ w1                 [192, 512]  (local)
           P_c = Bx[:, vsl_c] @ w1[vsl_c]       [1536, 512] (partial
               + A[:, usl_c] @ M_c                            sums)
           prow_c = gpe_b[usl_c] @ M_c + gpi_b[vsl_c] @ w1[vsl_c]
  AllReduce over cores: Wfold = sum_c P_c ; bias row = sum_c prow_c.

The AllReduce is split into two h-halves so the batch pass starts on
h-columns 0:256 while the second half is still on the wire.  The
batch pass keeps the stationary operand fixed across the 4 batch
tiles (i-outer, t-inner) to amortize LDWEIGHTS, 4 PSUM banks wide.
gpsimd queue carries only collectives; sync carries the
dependency-laden loads; scalar carries free-flowing streams (x).

Host prep is layout/dtype only (no FLOPs): bf16 casts, transposes of
x/gpe_w/gpi_w, row/column slicing and an even/odd interleave
permutation of each 192-row slice (so the two 96-row PE groups are
contiguous and drains are single DMAs).
"""

import numpy as np
import ml_dtypes

BF = ml_dtypes.bfloat16

NCORES = 8
B = 16384
BS = B // NCORES          # 2048 rows per core
BT = 512                  # batch tile (matmul free dim)
NBT = BS // BT            # 4
D1 = 1536                 # gpe input dim (x features)
H = 512                   # mlp hidden
HH = H // 2               # 256: AllReduce column half
A = 6                     # action dim
SL = D1 // NCORES         # 192: fold rows per core
HSL = SL // 2             # 96: interleaved half-slice
PR = D1 + 1               # AllReduce rows: 1536 Wfold + 1 bias row

NI = D1 // 128            # 12 i-chunks (x features)
NV = D1 // 128            # 12 v-chunks (gpi outputs)
NH = H // 128             # 4 h-chunks (mlp hidden)

_CACHE = {}


def _build():
    import concourse.bacc as bacc
    import concourse.tile as tile
    from concourse import mybir

    FP32 = mybir.dt.float32
    BF16 = mybir.dt.bfloat16
    Act = mybir.ActivationFunctionType

    nc = bacc.Bacc(None, num_devices=NCORES)

    xT_d = nc.dram_tensor("xT", [D1, BS], BF16, kind="ExternalInput")
    # [1536, 384] = [masked uslice cols | wT uslice cols], interleave-permuted
    gpiu_d = nc.dram_tensor("gpiu", [D1, 2 * SL], BF16, kind="ExternalInput")
    # [192, 3072] = [mask | wT] rows vsl (BxT) / usl (AT), interleave-permuted
    bxp_d = nc.dram_tensor("bxp", [SL, 2 * D1], BF16, kind="ExternalInput")
    ap_d = nc.dram_tensor("apk", [SL, 2 * D1], BF16, kind="ExternalInput")
    w1_d = nc.dram_tensor("w1", [D1, H], BF16, kind="ExternalInput")
    w1vs_d = nc.dram_tensor("w1vs", [SL, H], BF16, kind="ExternalInput")
    w2_d = nc.dram_tensor("w2", [H, H], BF16, kind="ExternalInput")
    w3_d = nc.dram_tensor("w3", [H, A], BF16, kind="ExternalInput")
    gpebp_d = nc.dram_tensor("gpebp", [HSL, 2], FP32, kind="ExternalInput")
    gpibp_d = nc.dram_tensor("gpibp", [HSL, 2], FP32, kind="ExternalInput")
    b1_d = nc.dram_tensor("b1", [H], FP32, kind="ExternalInput")
    b2_d = nc.dram_tensor("b2", [H], FP32, kind="ExternalInput")
    b3_d = nc.dram_tensor("b3", [A], FP32, kind="ExternalInput")
    o_d = nc.dram_tensor("out", [A, BS], FP32, kind="ExternalOutput")

    RG = [list(range(NCORES))]

    with tile.TileContext(nc) as tc:
        with (
            tc.tile_pool(name="wp", bufs=1) as wp,           # persistent
            tc.tile_pool(name="pc", bufs=1) as pcp,          # P drain staging
            tc.tile_pool(name="ap", bufs=1) as ap,           # activations
            tc.tile_pool(name="dp", bufs=1, space="DRAM") as dp,
            tc.tile_pool(name="psp", bufs=8, space="PSUM") as psp,
        ):
            def ps_tile():
                return psp.tile([128, BT], FP32, tag="ps", name="ps")

            # ---- fold operand stream: gpi uslice + w1 (F1s), alternating
            gpiu = []
            w1t = []
            for v in range(NV):
                q = nc.sync if (v % 2 == 0) else nc.scalar
                m = wp.tile([128, 2 * SL], BF16, tag=f"gpiu{v}")
                q.dma_start(out=m[:, :], in_=gpiu_d[v * 128:(v + 1) * 128, :])
                w = wp.tile([128, H], BF16, tag=f"w1_{v}")
                q.dma_start(out=w[:, :], in_=w1_d[v * 128:(v + 1) * 128, :])
                nc.vector.tensor_mul(m[:, 0:SL], m[:, 0:SL], m[:, SL:2 * SL])
                gpiu.append(m)
                w1t.append(w)

            # ---- partial-P operands: BxT/AT row-slices, w1 row-slice
            bxp, apk, w1vs = [], [], []
            for g in range(2):
                t = wp.tile([HSL, 2 * D1], BF16, tag=f"bxp{g}")
                nc.sync.dma_start(out=t[:, :],
                                  in_=bxp_d[g * HSL:(g + 1) * HSL, :])
                nc.vector.tensor_mul(t[:, 0:D1], t[:, 0:D1], t[:, D1:2 * D1])
                bxp.append(t)
                t = wp.tile([HSL, 2 * D1], BF16, tag=f"apk{g}")
                nc.scalar.dma_start(out=t[:, :],
                                    in_=ap_d[g * HSL:(g + 1) * HSL, :])
                nc.vector.tensor_mul(t[:, 0:D1], t[:, 0:D1], t[:, D1:2 * D1])
                apk.append(t)
                t = wp.tile([HSL, H], BF16, tag=f"w1vs{g}")
                nc.sync.dma_start(out=t[:, :],
                                  in_=w1vs_d[g * HSL:(g + 1) * HSL, :])
                w1vs.append(t)

            # small loads
            gpebp = wp.tile([HSL, 2], FP32, tag="gpebp")
            nc.scalar.dma_start(out=gpebp[:, :], in_=gpebp_d[:, :])
            gpibp = wp.tile([HSL, 2], FP32, tag="gpibp")
            nc.scalar.dma_start(out=gpibp[:, :], in_=gpibp_d[:, :])
            gpebf = wp.tile([HSL, 2], BF16, tag="gpebf")
            nc.vector.tensor_copy(gpebf[:, :], gpebp[:, :])
            gpibf = wp.tile([HSL, 2], BF16, tag="gpibf")
            nc.vector.tensor_copy(gpibf[:, :], gpibp[:, :])
            b2_sb = wp.tile([128, NH], FP32, tag="b2sb")
            nc.scalar.dma_start(out=b2_sb[:, :],
                                in_=b2_d.rearrange("(c p) -> p c", p=128))
            b3_sb = wp.tile([A, 1], FP32, tag="b3sb")
            nc.scalar.dma_start(out=b3_sb[:, :],
                                in_=b3_d.rearrange("(a one) -> a one", one=1))
            b1row = wp.tile([1, H], FP32, tag="b1row")
            nc.scalar.dma_start(out=b1row[:, :],
                                in_=b1_d.rearrange("(one h) -> one h", one=1))
            w2t = []
            for k in range(NH):
                t = wp.tile([128, H], BF16, tag=f"w2_{k}")
                nc.scalar.dma_start(out=t[:, :], in_=w2_d[k * 128:(k + 1) * 128, :])
                w2t.append(t)
            w3t = []
            for k in range(NH):
                t = wp.tile([128, A], BF16, tag=f"w3_{k}")
                nc.scalar.dma_start(out=t[:, :], in_=w3_d[k * 128:(k + 1) * 128, :])
                w3t.append(t)

            # ---- F1s: local M slice, two interleaved 96-row groups ->
            # msb[:, g*512:(g+1)*512] holds M rows {2p+g} in bf16
            ps_m = [ps_tile() for _ in range(2)]
            for v in range(NV):
                for g in range(2):
                    nc.tensor.matmul(ps_m[g][0:HSL, :],
                                     gpiu[v][:, g * HSL:(g + 1) * HSL],
                                     w1t[v][:, :],
                                     start=(v == 0), stop=(v == NV - 1))
            msb = wp.tile([HSL, 2 * H], BF16, tag="msb")
            for g in range(2):
                nc.vector.tensor_copy(msb[:, g * H:(g + 1) * H],
                                      ps_m[g][0:HSL, :])

            # ---- partial P chunks, drained pair-wise: two 128-row chunks
            # share one [128, 1024] staging tile, stored as one big DMA per
            # column half (a-halves on sync, b-halves on scalar)
            pa_dram = dp.tile([PR, HH], BF16, tag="pa_d")
            pb_dram = dp.tile([PR, HH], BF16, tag="pb_d")
            for ip in range(NI // 2):
                sb = pcp.tile([128, 2 * BT], BF16, tag=f"pcs{ip}")
                for ci in range(2):
                    i = 2 * ip + ci
                    ps = ps_tile()
                    for g in range(2):
                        nc.tensor.matmul(ps[:, :],
                                         bxp[g][:, i * 128:(i + 1) * 128],
                                         w1vs[g][:, :],
                                         start=(g == 0), stop=False)
                    for g in range(2):
                        nc.tensor.matmul(ps[:, :],
                                         apk[g][:, i * 128:(i + 1) * 128],
                                         msb[:, g * H:(g + 1) * H],
                                         start=False, stop=(g == 1))
                    nc.vector.tensor_copy(sb[:, ci * BT:(ci + 1) * BT],
                                          ps[:, :])
                sbv = sb[:, :].rearrange("p (c q) -> p c q", c=2)
                nc.sync.dma_start(
                    out=pa_dram[ip * 256:(ip + 1) * 256, :].rearrange(
                        "(c p) h -> p c h", c=2),
                    in_=sbv[:, :, 0:HH])
                nc.scalar.dma_start(
                    out=pb_dram[ip * 256:(ip + 1) * 256, :].rearrange(
                        "(c p) h -> p c h", c=2),
                    in_=sbv[:, :, HH:2 * HH])

            # bias partial row (row 1536 of the AllReduce payload)
            psb = ps_tile()
            for g in range(2):
                nc.tensor.matmul(psb[0:1, :], gpibf[:, g:g + 1], w1vs[g][:, :],
                                 start=(g == 0), stop=False)
            for g in range(2):
                nc.tensor.matmul(psb[0:1, :], gpebf[:, g:g + 1],
                                 msb[:, g * H:(g + 1) * H],
                                 start=False, stop=(g == 1))
            prow = wp.tile([1, H], BF16, tag="prow")
            nc.vector.tensor_copy(prow[:, :], psb[0:1, :])
            nc.sync.dma_start(out=pa_dram[D1:PR, :], in_=prow[:, 0:HH])
            nc.scalar.dma_start(out=pb_dram[D1:PR, :], in_=prow[:, HH:2 * HH])

            # ---- ONE AllReduce, split into two h-halves
            wfa_dram = dp.tile([PR, HH], BF16, tag="wfa_d", addr_space="Shared")
            wfb_dram = dp.tile([PR, HH], BF16, tag="wfb_d", addr_space="Shared")
            nc.gpsimd.collective_compute(
                "AllReduce", mybir.AluOpType.add, replica_groups=RG,
                ins=[pa_dram[:, :].opt()], outs=[wfa_dram[:, :].opt()])
            nc.gpsimd.collective_compute(
                "AllReduce", mybir.AluOpType.add, replica_groups=RG,
                ins=[pb_dram[:, :].opt()], outs=[wfb_dram[:, :].opt()])

            # ---- x tiles stream on scalar meanwhile
            xt = [[None] * NI for _ in range(NBT)]
            for t_i in range(NBT):
                for i in range(NI):
                    t = wp.tile([128, BT], BF16, tag=f"x{t_i}_{i}")
                    nc.scalar.dma_start(out=t[:, :],
                                        in_=xT_d[i * 128:(i + 1) * 128,
                                                 t_i * BT:(t_i + 1) * BT])
                    xt[t_i][i] = t

            # ---- Wfold reloads: bias row first, then a-halves (gated on
            # AR a), then b-halves
            browb = wp.tile([1, H], BF16, tag="browb")
            nc.sync.dma_start(out=browb[:, 0:HH], in_=wfa_dram[D1:PR, :])
            Wf = []
            for i in range(NI):
                t = wp.tile([128, H], BF16, tag=f"Wf{i}")
                nc.sync.dma_start(out=t[:, 0:HH],
                                  in_=wfa_dram[i * 128:(i + 1) * 128, :])
                Wf.append(t)
            nc.sync.dma_start(out=browb[:, HH:H], in_=wfb_dram[D1:PR, :])
            for i in range(NI):
                nc.sync.dma_start(out=Wf[i][:, HH:H],
                                  in_=wfb_dram[i * 128:(i + 1) * 128, :])

            # bias row + b1, transposed [1,512] -> [128,4] via a DRAM
            # roundtrip on the (idle) gpsimd queue, one half per AllReduce
            # so hc 0/1 activations don't wait on AR b.
            brow = wp.tile([1, H], FP32, tag="brow")
            brow_dram = dp.tile([1, H], FP32, tag="brow_d")
            bfold = wp.tile([128, NH], FP32, tag="bfold")
            for half in range(2):
                lo, hi = half * HH, (half + 1) * HH
                nc.vector.tensor_add(brow[:, lo:hi], browb[:, lo:hi],
                                     b1row[:, lo:hi])
                nc.gpsimd.dma_start(out=brow_dram[:, lo:hi],
                                    in_=brow[:, lo:hi])
                nc.gpsimd.dma_start(
                    out=bfold[:, half * 2:(half + 1) * 2],
                    in_=brow_dram[:, lo:hi].rearrange(
                        "one (c p) -> p (one c)", p=128))

            # ---- batch pass: 4 tiles of 512 rows, stationary reused across
            # tiles (i-outer, t-inner), 4 PSUM banks wide
            h1 = [[None] * NH for _ in range(NBT)]
            for hc in range(NH):
                ps1 = [ps_tile() for _ in range(NBT)]
                for i in range(NI):
                    for t_i in range(NBT):
                        nc.tensor.matmul(ps1[t_i][:, :],
                                         Wf[i][:, hc * 128:(hc + 1) * 128],
                                         xt[t_i][i][:, :],
                                         start=(i == 0), stop=(i == NI - 1))
                for t_i in range(NBT):
                    h = ap.tile([128, BT], BF16, tag=f"h1_{t_i}_{hc}")
                    nc.scalar.activation(h[:, :], ps1[t_i][:, :], Act.Relu,
                                         bias=bfold[:, hc:hc + 1])
                    h1[t_i][hc] = h

            h2 = [[None] * NH for _ in range(NBT)]
            for mc in range(NH):
                ps2 = [ps_tile() for _ in range(NBT)]
                for k in range(NH):
                    for t_i in range(NBT):
                        nc.tensor.matmul(ps2[t_i][:, :],
                                         w2t[k][:, mc * 128:(mc + 1) * 128],
                                         h1[t_i][k][:, :],
                                         start=(k == 0), stop=(k == NH - 1))
                for t_i in range(NBT):
                    h = ap.tile([128, BT], BF16, tag=f"h2_{t_i}_{mc}")
                    nc.scalar.activation(h[:, :], ps2[t_i][:, :], Act.Relu,
                                         bias=b2_sb[:, mc:mc + 1])
                    h2[t_i][mc] = h

            # L3 per-tile so each output store overlaps the next tile's matmuls
            for t_i in range(NBT):
                pso = ps_tile()
                for k in range(NH):
                    nc.tensor.matmul(pso[0:A, :], w3t[k][:, :],
                                     h2[t_i][k][:, :],
                                     start=(k == 0), stop=(k == NH - 1))
                osb = ap.tile([A, BT], FP32, tag=f"osb{t_i}")
                nc.scalar.activation(osb[:, :], pso[0:A, :], Act.Relu,
                                     bias=b3_sb[:, 0:1])
                nc.sync.dma_start(out=o_d[:, t_i * BT:(t_i + 1) * BT],
                                  in_=osb[:, :])

    nc.finalize()
    return nc


def _get_nc():
    if "nc" not in _CACHE:
        _CACHE["nc"] = _build()
    return _CACHE["nc"]


def _prep_inputs(inputs):
    """Host-side layout/dtype prep only (no network FLOPs): bf16 casts,
    transposes, per-core row/column slicing and interleave permutation."""
    f = {k: np.asarray(v) for k, v in inputs.items()}
    xT = np.ascontiguousarray(f["x"].astype(BF).T)            # [1536, B]
    gpem = f["gpe_mask"].astype(BF)                           # [u, i]
    gpewT = np.ascontiguousarray(f["gpe_w"].astype(BF).T)     # [u, i]
    gpim = f["gpi_mask"].astype(BF)                           # [v, j]
    gpiwT = np.ascontiguousarray(f["gpi_w"].astype(BF).T)     # [v, j]
    w1 = f["w1"].astype(BF)
    gpe_b = np.asarray(f["gpe_b"], dtype=np.float32)
    gpi_b = np.asarray(f["gpi_b"], dtype=np.float32)
    # packed position g*96 + p  <->  logical slice index 2p + g
    perm = np.concatenate([np.arange(0, SL, 2), np.arange(1, SL, 2)])
    shared = {
        "w1": np.ascontiguousarray(w1),
        "w2": np.ascontiguousarray(f["w2"].astype(BF)),
        "w3": np.ascontiguousarray(f["w3"].astype(BF)),
        "b1": np.ascontiguousarray(f["b1"], dtype=np.float32),
        "b2": np.ascontiguousarray(f["b2"], dtype=np.float32),
        "b3": np.ascontiguousarray(f["b3"], dtype=np.float32),
    }
    in_maps = []
    for c in range(NCORES):
        sl = np.arange(c * SL, (c + 1) * SL)[perm]   # permuted local slice
        usl = D1 + sl                                # gpi columns for u-part
        in_maps.append(dict(
            shared,
            xT=np.ascontiguousarray(xT[:, c * BS:(c + 1) * BS]),
            gpiu=np.ascontiguousarray(
                np.concatenate([gpim[:, usl], gpiwT[:, usl]], axis=1)),
            bxp=np.ascontiguousarray(
                np.concatenate([gpim[sl][:, :D1], gpiwT[sl][:, :D1]], axis=1)),
            apk=np.ascontiguousarray(
                np.concatenate([gpem[sl], gpewT[sl]], axis=1)),
            w1vs=np.ascontiguousarray(w1[sl]),
            gpebp=np.ascontiguousarray(
                gpe_b[sl].reshape(2, HSL).T, dtype=np.float32),
            gpibp=np.ascontiguousarray(
                gpi_b[sl].reshape(2, HSL).T, dtype=np.float32),
        ))
    return in_maps


def _run(inputs, trace=False):
    from concourse.bass_utils import run_bass_kernel_spmd

    nc = _get_nc()
    in_maps = _prep_inputs(inputs)
    res = run_bass_kernel_spmd(nc, in_maps, list(range(NCORES)), trace=trace)
    out = np.concatenate(
        [np.asarray(res.results[c]["out"]).T for c in range(NCORES)], axis=0)
    return out.astype(np.float32), res


def kernel(**inputs):
    out, _ = _run(inputs, trace=False)
    return out
